# revision 1
# baseline (speedup 1.0000x reference)
"""LoRA linear layer on 8 Trainium2 NeuronCores.

Computes y = x @ W^T + b + 2.0 * (x @ A^T) @ B^T for
x:[4,4096,1024], W:[1024,1024], b:[1024], A:[16,1024], B:[1024,16].

Host side folds the LoRA update into the weight (W_eff = W + 2*B@A, an exact
algebraic identity), so the device kernel is a single GEMM + bias. Sharding is
data-parallel over the 16384 tokens: each of the 8 cores computes a
[2048, 1024] output slice with replicated weights.

Device kernel (per core): y_c[m,o] = sum_d xT_c[d,m] * WeffT[d,o] + b[o]
  - xT_c  [1024, 2048] f32 (host-transposed so the contraction dim d lands on
    SBUF partitions for both matmul operands)
  - WeffT [1024, 1024] f32, fully resident in SBUF
  - float32r matmuls (full PE rate at N=512), fp32 PSUM accumulation
  - bias broadcast to 128 partitions on host; fused add on the DVE during
    PSUM->SBUF eviction
"""

import numpy as np

import concourse.mybir as mybir
import concourse.tile as tile
from concourse import bacc
from concourse.bass_utils import run_bass_kernel_spmd

N_CORES = 8
P = 128
D = 1024  # in_features (contraction)
O = 1024  # out_features
M_TOTAL = 4 * 4096  # tokens
M = M_TOTAL // N_CORES  # tokens per core
KO = D // P  # k-subtiles
SC = 512  # m super-chunk (DMA granularity)
SCALING = 2.0

# Set by test harnesses to capture profiling info; harmless otherwise.
TRACE = False
LAST_RESULT = None

_NC_CACHE = None


def _build_nc():
    f32 = mybir.dt.float32
    f32r = mybir.dt.float32r

    nc = bacc.Bacc("TRN2", debug=False)
    xT = nc.dram_tensor("xT", [D, M], f32r, kind="ExternalInput")
    wT = nc.dram_tensor("wT", [D, O], f32r, kind="ExternalInput")
    bias = nc.dram_tensor("bias", [P, O], f32, kind="ExternalInput")
    y = nc.dram_tensor("y", [M, O], f32, kind="ExternalOutput")

    xT_v = xT[:].rearrange("(ko p) m -> p ko m", p=P)  # [128, 8, 2048]
    wT_v = wT[:].rearrange("(ko p) o -> p ko o", p=P)  # [128, 8, 1024]
    y_v = y[:].rearrange("(mt p) o -> p mt o", p=P)  # [128, 16, 1024]

    with tile.TileContext(nc) as tc:
        with (
            tc.tile_pool(name="wpool", bufs=1) as wpool,
            tc.tile_pool(name="bpool", bufs=1) as bpool,
            tc.tile_pool(name="xpool", bufs=2) as xpool,
            tc.tile_pool(name="opool", bufs=3) as opool,
            tc.tile_pool(name="psum", bufs=2, space="PSUM") as psum,
        ):
            wt = wpool.tile([P, KO, O], f32r)
            nc.sync.dma_start(wt[:], wT_v)
            bt = bpool.tile([P, O], f32)
            nc.sync.dma_start(bt[:], bias[:])

            for sc in range(M // SC):
                xt = xpool.tile([P, KO, SC], f32r, tag="xt")
                nc.sync.dma_start(xt[:], xT_v[:, :, sc * SC : (sc + 1) * SC])
                for mt_i in range(SC // P):
                    mt = sc * (SC // P) + mt_i
                    ps = psum.tile([P, 2, 512], mybir.dt.float32, tag="ps")
                    for ko in range(KO):
                        first, last = ko == 0, ko == KO - 1
                        lhsT = xt[:, ko, mt_i * P : (mt_i + 1) * P]
                        for half in range(2):
                            nc.tensor.matmul(
                                ps[:, half, :],
                                lhsT,
                                wt[:, ko, half * 512 : (half + 1) * 512],
                                start=first,
                                stop=last,
                            )
                    ot = opool.tile([P, O], f32, tag="ot")
                    nc.vector.tensor_tensor(
                        ot[:],
                        ps[:].rearrange("p a b -> p (a b)"),
                        bt[:],
                        mybir.AluOpType.add,
                    )
                    nc.sync.dma_start(y_v[:, mt, :], ot[:])

    nc.compile()
    return nc


def _get_nc():
    global _NC_CACHE
    if _NC_CACHE is None:
        _NC_CACHE = _build_nc()
    return _NC_CACHE


def kernel(x, W, b, A, B):
    global LAST_RESULT
    x = np.ascontiguousarray(np.asarray(x, dtype=np.float32))
    W = np.asarray(W, dtype=np.float32)
    b = np.asarray(b, dtype=np.float32)
    A = np.asarray(A, dtype=np.float32)
    B = np.asarray(B, dtype=np.float32)
    assert x.shape == (4, 4096, D) and W.shape == (O, D)
    assert b.shape == (O,) and A.shape[1] == D and B.shape[0] == O

    # Fold the LoRA update into the weight: x@W^T + s*(x@A^T)@B^T = x@(W + s*B@A)^T
    Weff = (
        W.astype(np.float64) + SCALING * (B.astype(np.float64) @ A.astype(np.float64))
    ).astype(np.float32)
    WeffT = np.ascontiguousarray(Weff.T)  # [D, O]
    bias_rep = np.ascontiguousarray(np.broadcast_to(b[None, :], (P, O)))

    xr = x.reshape(M_TOTAL, D)
    in_maps = []
    for c in range(N_CORES):
        xTc = np.ascontiguousarray(xr[c * M : (c + 1) * M].T)  # [D, M]
        in_maps.append({"xT": xTc, "wT": WeffT, "bias": bias_rep})

    nc = _get_nc()
    res = run_bass_kernel_spmd(
        nc, in_maps, core_ids=list(range(N_CORES)), trace=TRACE
    )
    LAST_RESULT = res

    out = np.concatenate([res.results[c]["y"] for c in range(N_CORES)], axis=0)
    return out.reshape(x.shape[0], x.shape[1], O)


# revision 2
# speedup vs baseline: 1.0219x; 1.0219x over previous
"""LoRA linear layer on 8 Trainium2 NeuronCores.

Computes y = x @ W^T + b + 2.0 * (x @ A^T) @ B^T for
x:[4,4096,1024], W:[1024,1024], b:[1024], A:[16,1024], B:[1024,16].

Host side folds the LoRA update into the weight (W_eff = W + 2*B@A, an exact
algebraic identity), so the device kernel is a single GEMM + bias. Sharding is
data-parallel over the 16384 tokens: each of the 8 cores computes a
[2048, 1024] output slice with replicated weights.

Device kernel (per core): y_c[m,o] = sum_d xT_c[d,m] * WeffT[d,o] + b[o]
  - xT_c  [1024, 2048] f32 (host-transposed so the contraction dim d lands on
    SBUF partitions for both matmul operands)
  - WeffT [1024, 1024] f32, fully resident in SBUF
  - float32r matmuls (full PE rate at N=512), fp32 PSUM accumulation
  - bias broadcast to 128 partitions on host; fused add on the DVE during
    PSUM->SBUF eviction
"""

import numpy as np

import concourse.mybir as mybir
import concourse.tile as tile
from concourse import bacc
from concourse.bass_utils import run_bass_kernel_spmd

N_CORES = 8
P = 128
D = 1024  # in_features (contraction)
O = 1024  # out_features
M_TOTAL = 4 * 4096  # tokens
M = M_TOTAL // N_CORES  # tokens per core
KO = D // P  # k-subtiles
SC = 512  # m super-chunk (DMA granularity)
SCALING = 2.0

# Set by test harnesses to capture profiling info; harmless otherwise.
TRACE = False
LAST_RESULT = None

_NC_CACHE = None


def _build_nc():
    f32 = mybir.dt.float32
    f32r = mybir.dt.float32r

    nc = bacc.Bacc("TRN2", debug=False)
    xT = nc.dram_tensor("xT", [D, M], f32r, kind="ExternalInput")
    wT = nc.dram_tensor("wT", [D, O], f32r, kind="ExternalInput")
    bias = nc.dram_tensor("bias", [P, O], f32, kind="ExternalInput")
    y = nc.dram_tensor("y", [M, O], f32, kind="ExternalOutput")

    xT_v = xT[:].rearrange("(ko p) m -> p ko m", p=P)  # [128, 8, 2048]
    wT_v = wT[:].rearrange("(ko p) o -> p ko o", p=P)  # [128, 8, 1024]
    y_v = y[:].rearrange("(mt p) o -> p mt o", p=P)  # [128, 16, 1024]

    n_sc = M // SC
    with tile.TileContext(nc) as tc:
        with (
            tc.tile_pool(name="wpool", bufs=1) as wpool,
            tc.tile_pool(name="bpool", bufs=1) as bpool,
            tc.tile_pool(name="xpool", bufs=4) as xpool,
            tc.tile_pool(name="opool", bufs=3) as opool,
            tc.tile_pool(name="psum", bufs=2, space="PSUM") as psum,
        ):
            # x super-chunks arrive as two half-K tiles so the first matmuls
            # only wait on 1 MiB; issued before W so the PE can start early.
            xh = {}

            def load_x(sc):
                for h in range(2):
                    t = xpool.tile([P, KO // 2, SC], f32r, tag="xt")
                    nc.sync.dma_start(
                        t[:],
                        xT_v[
                            :,
                            (KO // 2) * h : (KO // 2) * (h + 1),
                            sc * SC : (sc + 1) * SC,
                        ],
                    )
                    xh[(sc, h)] = t

            load_x(0)
            # W split per k-subtile: the ko-th group of matmuls only depends
            # on its own 512 KiB slice, so compute ramps while W streams in.
            wt = []
            for ko in range(KO):
                t = wpool.tile([P, O], f32r, tag=f"w{ko}")
                nc.sync.dma_start(t[:], wT_v[:, ko, :])
                wt.append(t)
            bt = bpool.tile([P, O], f32)
            nc.sync.dma_start(bt[:], bias[:])

            for sc in range(n_sc):
                if sc + 1 < n_sc:
                    load_x(sc + 1)
                for mt_i in range(SC // P):
                    mt = sc * (SC // P) + mt_i
                    ps = psum.tile([P, 2, 512], mybir.dt.float32, tag="ps")
                    for ko in range(KO):
                        first, last = ko == 0, ko == KO - 1
                        lhsT = xh[(sc, ko // (KO // 2))][
                            :, ko % (KO // 2), mt_i * P : (mt_i + 1) * P
                        ]
                        for half in range(2):
                            nc.tensor.matmul(
                                ps[:, half, :],
                                lhsT,
                                wt[ko][:, half * 512 : (half + 1) * 512],
                                start=first,
                                stop=last,
                            )
                    ot = opool.tile([P, O], f32, tag="ot")
                    nc.vector.tensor_tensor(
                        ot[:],
                        ps[:].rearrange("p a b -> p (a b)"),
                        bt[:],
                        mybir.AluOpType.add,
                    )
                    nc.sync.dma_start(y_v[:, mt, :], ot[:])

    nc.compile()
    return nc


def _get_nc():
    global _NC_CACHE
    if _NC_CACHE is None:
        _NC_CACHE = _build_nc()
    return _NC_CACHE


def kernel(x, W, b, A, B):
    global LAST_RESULT
    x = np.ascontiguousarray(np.asarray(x, dtype=np.float32))
    W = np.asarray(W, dtype=np.float32)
    b = np.asarray(b, dtype=np.float32)
    A = np.asarray(A, dtype=np.float32)
    B = np.asarray(B, dtype=np.float32)
    assert x.shape == (4, 4096, D) and W.shape == (O, D)
    assert b.shape == (O,) and A.shape[1] == D and B.shape[0] == O

    # Fold the LoRA update into the weight: x@W^T + s*(x@A^T)@B^T = x@(W + s*B@A)^T
    Weff = (
        W.astype(np.float64) + SCALING * (B.astype(np.float64) @ A.astype(np.float64))
    ).astype(np.float32)
    WeffT = np.ascontiguousarray(Weff.T)  # [D, O]
    bias_rep = np.ascontiguousarray(np.broadcast_to(b[None, :], (P, O)))

    xr = x.reshape(M_TOTAL, D)
    in_maps = []
    for c in range(N_CORES):
        xTc = np.ascontiguousarray(xr[c * M : (c + 1) * M].T)  # [D, M]
        in_maps.append({"xT": xTc, "wT": WeffT, "bias": bias_rep})

    nc = _get_nc()
    res = run_bass_kernel_spmd(
        nc, in_maps, core_ids=list(range(N_CORES)), trace=TRACE
    )
    LAST_RESULT = res

    out = np.concatenate([res.results[c]["y"] for c in range(N_CORES)], axis=0)
    return out.reshape(x.shape[0], x.shape[1], O)


# revision 5
# speedup vs baseline: 1.1013x; 1.0777x over previous
"""LoRA linear layer on 8 Trainium2 NeuronCores.

Computes y = x @ W^T + b + 2.0 * (x @ A^T) @ B^T for
x:[4,4096,1024], W:[1024,1024], b:[1024], A:[16,1024], B:[1024,16].

Host side folds the LoRA update into the weight (W_eff = W + 2*B@A, an exact
algebraic identity), so the device kernel is a single GEMM + bias. Sharding is
data-parallel over the 16384 tokens: each of the 8 cores computes a
[2048, 1024] output slice with replicated weights.

Device kernel (per core): y_c[m,o] = sum_d xT_c[d,m] * WeffT[d,o] + b[o]
  - xT_c  [1024, 2048] f32 (host-transposed so the contraction dim d lands on
    SBUF partitions for both matmul operands)
  - WeffT [1024, 1024] f32, fully resident in SBUF
  - float32r matmuls (full PE rate at N=512), fp32 PSUM accumulation
  - bias broadcast to 128 partitions on host; fused add on the DVE during
    PSUM->SBUF eviction
"""

import numpy as np

import concourse.mybir as mybir
import concourse.tile as tile
from concourse import bacc
from concourse.bass_utils import run_bass_kernel_spmd

N_CORES = 8
P = 128
D = 1024  # in_features (contraction)
O = 1024  # out_features
M_TOTAL = 4 * 4096  # tokens
M = M_TOTAL // N_CORES  # tokens per core
KO = D // P  # k-subtiles
SC = 512  # m super-chunk (DMA granularity)
SCALING = 2.0

# Set by test harnesses to capture profiling info; harmless otherwise.
TRACE = False
LAST_RESULT = None

_NC_CACHE = None


def _build_nc():
    f32 = mybir.dt.float32
    f32r = mybir.dt.float32r

    nc = bacc.Bacc("TRN2", debug=False)
    xT = nc.dram_tensor("xT", [D, M], f32r, kind="ExternalInput")
    wT = nc.dram_tensor("wT", [D, O], f32r, kind="ExternalInput")
    bias = nc.dram_tensor("bias", [P, O], f32, kind="ExternalInput")
    y = nc.dram_tensor("y", [M, O], f32, kind="ExternalOutput")

    xT_v = xT[:].rearrange("(ko p) m -> p ko m", p=P)  # [128, 8, 2048]
    wT_v = wT[:].rearrange("(ko p) o -> p ko o", p=P)  # [128, 8, 1024]
    y_v = y[:].rearrange("(mt p) o -> p mt o", p=P)  # [128, 16, 1024]

    n_sc = M // SC
    with tile.TileContext(nc) as tc:
        with (
            tc.tile_pool(name="wpool", bufs=1) as wpool,
            tc.tile_pool(name="bpool", bufs=1) as bpool,
            tc.tile_pool(name="xpool", bufs=4) as xpool,
            tc.tile_pool(name="opool", bufs=3) as opool,
            tc.tile_pool(name="psum", bufs=4, space="PSUM") as psum,
        ):
            # x super-chunks arrive as two half-K tiles; W split per k-subtile.
            # Issue order interleaves them (x_h0, W0-3, x_h1, W4-7) so the
            # first matmuls only wait on ~1.5 MiB and the ramp paces the
            # W stream instead of idling behind it.
            xh = {}

            def load_x_half(sc, h):
                t = xpool.tile([P, KO // 2, SC], f32r, tag="xt")
                nc.sync.dma_start(
                    t[:],
                    xT_v[
                        :,
                        (KO // 2) * h : (KO // 2) * (h + 1),
                        sc * SC : (sc + 1) * SC,
                    ],
                )
                xh[(sc, h)] = t

            wt = [None] * KO

            def load_w(ko):
                t = wpool.tile([P, O], f32r, tag=f"w{ko}")
                nc.sync.dma_start(t[:], wT_v[:, ko, :])
                wt[ko] = t

            load_x_half(0, 0)
            for ko in range(4):
                load_w(ko)
            load_x_half(0, 1)
            for ko in range(4, KO):
                load_w(ko)
            bt = bpool.tile([P, O], f32)
            nc.sync.dma_start(bt[:], bias[:])

            def x_slice(sc, ko, mt_i):
                return xh[(sc, ko // (KO // 2))][
                    :, ko % (KO // 2), mt_i * P : (mt_i + 1) * P
                ]

            def evict(ps, mt):
                ot = opool.tile([P, O], f32, tag="ot")
                nc.vector.tensor_tensor(
                    ot[:],
                    ps[:].rearrange("p a b -> p (a b)"),
                    bt[:],
                    mybir.AluOpType.add,
                )
                nc.gpsimd.dma_start(y_v[:, mt, :], ot[:])

            MPC = SC // P  # m-tiles per super-chunk

            # sc0: ko-outer so each W slice is consumed as it lands; all four
            # m-tiles accumulate simultaneously across the 8 PSUM banks.
            pss = [
                psum.tile([P, 2, 512], mybir.dt.float32, tag="ps", name=f"ps{i}")
                for i in range(MPC)
            ]
            for ko in range(KO):
                for mt_i in range(MPC):
                    for half in range(2):
                        nc.tensor.matmul(
                            pss[mt_i][:, half, :],
                            x_slice(0, ko, mt_i),
                            wt[ko][:, half * 512 : (half + 1) * 512],
                            start=ko == 0,
                            stop=ko == KO - 1,
                        )
            load_x_half(1, 0)
            load_x_half(1, 1)
            for mt_i in range(MPC):
                evict(pss[mt_i], mt_i)

            # sc1+: mt-outer with PSUM double buffering (steady state).
            for sc in range(1, n_sc):
                if sc + 1 < n_sc:
                    load_x_half(sc + 1, 0)
                    load_x_half(sc + 1, 1)
                for mt_i in range(MPC):
                    mt = sc * MPC + mt_i
                    ps = psum.tile([P, 2, 512], mybir.dt.float32, tag="ps")
                    for ko in range(KO):
                        for half in range(2):
                            nc.tensor.matmul(
                                ps[:, half, :],
                                x_slice(sc, ko, mt_i),
                                wt[ko][:, half * 512 : (half + 1) * 512],
                                start=ko == 0,
                                stop=ko == KO - 1,
                            )
                    evict(ps, mt)

    nc.compile()
    return nc


def _get_nc():
    global _NC_CACHE
    if _NC_CACHE is None:
        _NC_CACHE = _build_nc()
    return _NC_CACHE


def kernel(x, W, b, A, B):
    global LAST_RESULT
    x = np.ascontiguousarray(np.asarray(x, dtype=np.float32))
    W = np.asarray(W, dtype=np.float32)
    b = np.asarray(b, dtype=np.float32)
    A = np.asarray(A, dtype=np.float32)
    B = np.asarray(B, dtype=np.float32)
    assert x.shape == (4, 4096, D) and W.shape == (O, D)
    assert b.shape == (O,) and A.shape[1] == D and B.shape[0] == O

    # Fold the LoRA update into the weight: x@W^T + s*(x@A^T)@B^T = x@(W + s*B@A)^T
    Weff = (
        W.astype(np.float64) + SCALING * (B.astype(np.float64) @ A.astype(np.float64))
    ).astype(np.float32)
    WeffT = np.ascontiguousarray(Weff.T)  # [D, O]
    bias_rep = np.ascontiguousarray(np.broadcast_to(b[None, :], (P, O)))

    xr = x.reshape(M_TOTAL, D)
    in_maps = []
    for c in range(N_CORES):
        xTc = np.ascontiguousarray(xr[c * M : (c + 1) * M].T)  # [D, M]
        in_maps.append({"xT": xTc, "wT": WeffT, "bias": bias_rep})

    nc = _get_nc()
    res = run_bass_kernel_spmd(
        nc, in_maps, core_ids=list(range(N_CORES)), trace=TRACE
    )
    LAST_RESULT = res

    out = np.concatenate([res.results[c]["y"] for c in range(N_CORES)], axis=0)
    return out.reshape(x.shape[0], x.shape[1], O)


# revision 7
# speedup vs baseline: 1.1131x; 1.0107x over previous
"""LoRA linear layer on 8 Trainium2 NeuronCores.

Computes y = x @ W^T + b + 2.0 * (x @ A^T) @ B^T for
x:[4,4096,1024], W:[1024,1024], b:[1024], A:[16,1024], B:[1024,16].

Host side folds the LoRA update into the weight (W_eff = W + 2*B@A, an exact
algebraic identity), so the device kernel is a single GEMM + bias. Sharding is
data-parallel over the 16384 tokens: each of the 8 cores computes a
[2048, 1024] output slice with replicated weights.

Device kernel (per core): y_c[m,o] = sum_d xT_c[d,m] * WeffT[d,o] + b[o]
  - xT_c  [1024, 2048] f32 (host-transposed so the contraction dim d lands on
    SBUF partitions for both matmul operands)
  - WeffT [1024, 1024] f32, fully resident in SBUF
  - float32r matmuls (full PE rate at N=512), fp32 PSUM accumulation
  - bias broadcast to 128 partitions on host; fused add on the DVE during
    PSUM->SBUF eviction
"""

import numpy as np

import concourse.mybir as mybir
import concourse.tile as tile
from concourse import bacc
from concourse.bass_utils import run_bass_kernel_spmd

N_CORES = 8
P = 128
D = 1024  # in_features (contraction)
O = 1024  # out_features
M_TOTAL = 4 * 4096  # tokens
M = M_TOTAL // N_CORES  # tokens per core
KO = D // P  # k-subtiles
SC = 512  # m super-chunk (DMA granularity)
SCALING = 2.0

# Set by test harnesses to capture profiling info; harmless otherwise.
TRACE = False
LAST_RESULT = None

_NC_CACHE = None


def _build_nc():
    f32 = mybir.dt.float32
    f32r = mybir.dt.float32r

    nc = bacc.Bacc("TRN2", debug=False)
    xT = nc.dram_tensor("xT", [D, M], f32r, kind="ExternalInput")
    wT = nc.dram_tensor("wT", [D, O], f32r, kind="ExternalInput")
    bias = nc.dram_tensor("bias", [P, O], f32, kind="ExternalInput")
    y = nc.dram_tensor("y", [M, O], f32, kind="ExternalOutput")

    xT_v = xT[:].rearrange("(ko p) m -> p ko m", p=P)  # [128, 8, 2048]
    wT_v = wT[:].rearrange("(ko p) o -> p ko o", p=P)  # [128, 8, 1024]
    y_v = y[:].rearrange("(mt p) o -> p mt o", p=P)  # [128, 16, 1024]

    n_sc = M // SC
    with tile.TileContext(nc) as tc:
        with (
            tc.tile_pool(name="wpool", bufs=1) as wpool,
            tc.tile_pool(name="bpool", bufs=1) as bpool,
            tc.tile_pool(name="xpool", bufs=4) as xpool,
            tc.tile_pool(name="opool", bufs=3) as opool,
            tc.tile_pool(name="psum", bufs=4, space="PSUM") as psum,
        ):
            # x super-chunks arrive as two half-K tiles; W split per k-subtile.
            # Issue order interleaves them (x_h0, W0-3, x_h1, W4-7) so the
            # first matmuls only wait on ~1.5 MiB and the ramp paces the
            # W stream instead of idling behind it.
            xh = {}

            def load_x_half(sc, h):
                t = xpool.tile([P, KO // 2, SC], f32r, tag="xt")
                nc.sync.dma_start(
                    t[:],
                    xT_v[
                        :,
                        (KO // 2) * h : (KO // 2) * (h + 1),
                        sc * SC : (sc + 1) * SC,
                    ],
                )
                xh[(sc, h)] = t

            wt = [None] * KO

            def load_w(ko):
                t = wpool.tile([P, O], f32r, tag=f"w{ko}")
                nc.sync.dma_start(t[:], wT_v[:, ko, :])
                wt[ko] = t

            # Zero warmup tile: ~14 throwaway matmuls keep the PE busy while
            # the first x/W slices stream in, so the HAM clock-gate is warm
            # (2.4 GHz) by the time real matmuls start.
            zt = wpool.tile([P, 512], mybir.dt.bfloat16, tag="warm")
            nc.gpsimd.memset(zt[:], 0.0)
            wps = psum.tile([P, 2, 512], mybir.dt.float32, tag="ps", name="wps")
            for _ in range(14):
                nc.tensor.matmul(
                    wps[:, 0, :], zt[:, :P], zt[:], start=True, stop=True
                )

            # sc0's x arrives per-ko (256 KiB) interleaved with W slices so the
            # first real matmul only waits on ~0.75 MiB.
            x0 = []
            for ko in range(KO):
                t = xpool.tile([P, SC], f32r, tag="x0", bufs=KO, name=f"x0_{ko}")
                nc.sync.dma_start(t[:], xT_v[:, ko, 0:SC])
                x0.append(t)
                load_w(ko)
            bt = bpool.tile([P, O], f32)
            nc.sync.dma_start(bt[:], bias[:])

            def x_slice(sc, ko, mt_i):
                if sc == 0:
                    return x0[ko][:, mt_i * P : (mt_i + 1) * P]
                return xh[(sc, ko // (KO // 2))][
                    :, ko % (KO // 2), mt_i * P : (mt_i + 1) * P
                ]

            def evict(ps, mt):
                ot = opool.tile([P, O], f32, tag="ot")
                nc.vector.tensor_tensor(
                    ot[:],
                    ps[:].rearrange("p a b -> p (a b)"),
                    bt[:],
                    mybir.AluOpType.add,
                )
                nc.gpsimd.dma_start(y_v[:, mt, :], ot[:])

            MPC = SC // P  # m-tiles per super-chunk

            # sc0: ko-outer so each W slice is consumed as it lands; all four
            # m-tiles accumulate simultaneously across the 8 PSUM banks.
            pss = [
                psum.tile([P, 2, 512], mybir.dt.float32, tag="ps", name=f"ps{i}")
                for i in range(MPC)
            ]
            for ko in range(KO):
                for mt_i in range(MPC):
                    for half in range(2):
                        nc.tensor.matmul(
                            pss[mt_i][:, half, :],
                            x_slice(0, ko, mt_i),
                            wt[ko][:, half * 512 : (half + 1) * 512],
                            start=ko == 0,
                            stop=ko == KO - 1,
                        )
            load_x_half(1, 0)
            load_x_half(1, 1)
            for mt_i in range(MPC):
                evict(pss[mt_i], mt_i)

            # sc1+: mt-outer with PSUM double buffering (steady state).
            for sc in range(1, n_sc):
                if sc + 1 < n_sc:
                    load_x_half(sc + 1, 0)
                    load_x_half(sc + 1, 1)
                for mt_i in range(MPC):
                    mt = sc * MPC + mt_i
                    ps = psum.tile([P, 2, 512], mybir.dt.float32, tag="ps")
                    for ko in range(KO):
                        for half in range(2):
                            nc.tensor.matmul(
                                ps[:, half, :],
                                x_slice(sc, ko, mt_i),
                                wt[ko][:, half * 512 : (half + 1) * 512],
                                start=ko == 0,
                                stop=ko == KO - 1,
                            )
                    evict(ps, mt)

    nc.compile()
    return nc


def _get_nc():
    global _NC_CACHE
    if _NC_CACHE is None:
        _NC_CACHE = _build_nc()
    return _NC_CACHE


def kernel(x, W, b, A, B):
    global LAST_RESULT
    x = np.ascontiguousarray(np.asarray(x, dtype=np.float32))
    W = np.asarray(W, dtype=np.float32)
    b = np.asarray(b, dtype=np.float32)
    A = np.asarray(A, dtype=np.float32)
    B = np.asarray(B, dtype=np.float32)
    assert x.shape == (4, 4096, D) and W.shape == (O, D)
    assert b.shape == (O,) and A.shape[1] == D and B.shape[0] == O

    # Fold the LoRA update into the weight: x@W^T + s*(x@A^T)@B^T = x@(W + s*B@A)^T
    Weff = (
        W.astype(np.float64) + SCALING * (B.astype(np.float64) @ A.astype(np.float64))
    ).astype(np.float32)
    WeffT = np.ascontiguousarray(Weff.T)  # [D, O]
    bias_rep = np.ascontiguousarray(np.broadcast_to(b[None, :], (P, O)))

    xr = x.reshape(M_TOTAL, D)
    in_maps = []
    for c in range(N_CORES):
        xTc = np.ascontiguousarray(xr[c * M : (c + 1) * M].T)  # [D, M]
        in_maps.append({"xT": xTc, "wT": WeffT, "bias": bias_rep})

    nc = _get_nc()
    res = run_bass_kernel_spmd(
        nc, in_maps, core_ids=list(range(N_CORES)), trace=TRACE
    )
    LAST_RESULT = res

    out = np.concatenate([res.results[c]["y"] for c in range(N_CORES)], axis=0)
    return out.reshape(x.shape[0], x.shape[1], O)


# revision 10
# speedup vs baseline: 1.1154x; 1.0021x over previous
"""LoRA linear layer on 8 Trainium2 NeuronCores.

Computes y = x @ W^T + b + 2.0 * (x @ A^T) @ B^T for
x:[4,4096,1024], W:[1024,1024], b:[1024], A:[16,1024], B:[1024,16].

Host side folds the LoRA update into the weight (W_eff = W + 2*B@A, an exact
algebraic identity), so the device kernel is a single GEMM + bias. Sharding is
data-parallel over the 16384 tokens: each of the 8 cores computes a
[2048, 1024] output slice with replicated weights.

Device kernel (per core): y_c[m,o] = sum_d xT_c[d,m] * WeffT[d,o] + b[o]
  - xT_c  [1024, 2048] f32 (host-transposed so the contraction dim d lands on
    SBUF partitions for both matmul operands)
  - WeffT [1024, 1024] f32, fully resident in SBUF
  - float32r matmuls (full PE rate at N=512), fp32 PSUM accumulation
  - bias broadcast to 128 partitions on host; fused add on the DVE during
    PSUM->SBUF eviction
"""

import numpy as np

import concourse.mybir as mybir
import concourse.tile as tile
from concourse import bacc
from concourse.bass_utils import run_bass_kernel_spmd

N_CORES = 8
P = 128
D = 1024  # in_features (contraction)
O = 1024  # out_features
M_TOTAL = 4 * 4096  # tokens
M = M_TOTAL // N_CORES  # tokens per core
KO = D // P  # k-subtiles
SC = 512  # m super-chunk (DMA granularity)
SCALING = 2.0

# Set by test harnesses to capture profiling info; harmless otherwise.
TRACE = False
LAST_RESULT = None

_NC_CACHE = None


def _build_nc():
    f32 = mybir.dt.float32
    f32r = mybir.dt.float32r

    nc = bacc.Bacc("TRN2", debug=False)
    xT = nc.dram_tensor("xT", [D, M], f32r, kind="ExternalInput")
    wT = nc.dram_tensor("wT", [D, O], f32r, kind="ExternalInput")
    bias = nc.dram_tensor("bias", [P, O], f32, kind="ExternalInput")
    y = nc.dram_tensor("y", [M, O], f32, kind="ExternalOutput")

    xT_v = xT[:].rearrange("(ko p) m -> p ko m", p=P)  # [128, 8, 2048]
    wT_v = wT[:].rearrange("(ko p) o -> p ko o", p=P)  # [128, 8, 1024]
    y_v = y[:].rearrange("(mt p) o -> p mt o", p=P)  # [128, 16, 1024]

    n_sc = M // SC
    with tile.TileContext(nc) as tc:
        with (
            tc.tile_pool(name="wpool", bufs=1) as wpool,
            tc.tile_pool(name="bpool", bufs=1) as bpool,
            tc.tile_pool(name="xpool", bufs=4) as xpool,
            tc.tile_pool(name="opool", bufs=3) as opool,
            tc.tile_pool(name="psum", bufs=8, space="PSUM") as psum,
        ):
            # x super-chunks arrive as two half-K tiles; W split per k-subtile.
            # Issue order interleaves them (x_h0, W0-3, x_h1, W4-7) so the
            # first matmuls only wait on ~1.5 MiB and the ramp paces the
            # W stream instead of idling behind it.
            xh = {}

            def load_x_half(sc, h):
                t = xpool.tile([P, KO // 2, SC], f32r, tag="xt")
                nc.sync.dma_start(
                    t[:],
                    xT_v[
                        :,
                        (KO // 2) * h : (KO // 2) * (h + 1),
                        sc * SC : (sc + 1) * SC,
                    ],
                )
                xh[(sc, h)] = t

            wt = [None] * KO

            def load_w(ko):
                t = wpool.tile([P, O], f32r, tag=f"w{ko}")
                nc.sync.dma_start(t[:], wT_v[:, ko, :])
                wt[ko] = t

            # Zero warmup tile: ~14 throwaway matmuls keep the PE busy while
            # the first x/W slices stream in, so the HAM clock-gate is warm
            # (2.4 GHz) by the time real matmuls start.
            zt = wpool.tile([P, 512], mybir.dt.bfloat16, tag="warm")
            nc.gpsimd.memset(zt[:], 0.0)
            wps = psum.tile([P, 512], mybir.dt.float32, tag="ps", name="wps")
            for _ in range(12):
                nc.tensor.matmul(wps[:], zt[:, :P], zt[:], start=True, stop=True)

            # sc0's x arrives per-ko (256 KiB) interleaved with W slices so the
            # first real matmul only waits on ~0.75 MiB.
            x0 = []
            for ko in range(KO):
                t = xpool.tile([P, SC], f32r, tag="x0", bufs=KO, name=f"x0_{ko}")
                nc.sync.dma_start(t[:], xT_v[:, ko, 0:SC])
                x0.append(t)
                load_w(ko)
            bt = bpool.tile([P, O], f32)
            nc.sync.dma_start(bt[:], bias[:])

            def x_slice(sc, ko, mt_i):
                if sc == 0:
                    return x0[ko][:, mt_i * P : (mt_i + 1) * P]
                return xh[(sc, ko // (KO // 2))][
                    :, ko % (KO // 2), mt_i * P : (mt_i + 1) * P
                ]

            def evict_half(ps, ot, half):
                nc.vector.tensor_tensor(
                    ot[:, half * 512 : (half + 1) * 512],
                    ps[:],
                    bt[:, half * 512 : (half + 1) * 512],
                    mybir.AluOpType.add,
                )

            MPC = SC // P  # m-tiles per super-chunk

            # sc0: ko-outer so each W slice is consumed as it lands; all four
            # m-tiles accumulate simultaneously across the 8 PSUM banks
            # (one single-bank tile per (m-tile, output-half) group).
            pss = [
                [
                    psum.tile([P, 512], mybir.dt.float32, tag="ps", name=f"ps{i}_{h}")
                    for h in range(2)
                ]
                for i in range(MPC)
            ]
            ots0 = [opool.tile([P, O], f32, tag="ot", name=f"ot{i}") for i in range(MPC)]
            for ko in range(KO):
                last = ko == KO - 1
                for mt_i in range(MPC):
                    for half in range(2):
                        nc.tensor.matmul(
                            pss[mt_i][half][:],
                            x_slice(0, ko, mt_i),
                            wt[ko][:, half * 512 : (half + 1) * 512],
                            start=ko == 0,
                            stop=last,
                        )
                    if last:
                        # evict right behind each group's stop so PSUM slots
                        # recycle before the next super-chunk needs them
                        for half in range(2):
                            evict_half(pss[mt_i][half], ots0[mt_i], half)
            load_x_half(1, 0)
            load_x_half(1, 1)
            for mt_i in range(MPC):
                nc.gpsimd.dma_start(y_v[:, mt_i, :], ots0[mt_i][:])

            # sc1+: mt-outer, single-bank PSUM tiles rotating through 8 slots.
            for sc in range(1, n_sc):
                if sc + 1 < n_sc:
                    load_x_half(sc + 1, 0)
                    load_x_half(sc + 1, 1)
                for mt_i in range(MPC):
                    mt = sc * MPC + mt_i
                    final = sc == n_sc - 1 and mt_i == MPC - 1
                    ot = opool.tile([P, O], f32, tag="ot")
                    if not final:
                        ph = [
                            psum.tile([P, 512], mybir.dt.float32, tag="ps", name=f"ph{h}")
                            for h in range(2)
                        ]
                        for ko in range(KO):
                            for half in range(2):
                                nc.tensor.matmul(
                                    ph[half][:],
                                    x_slice(sc, ko, mt_i),
                                    wt[ko][:, half * 512 : (half + 1) * 512],
                                    start=ko == 0,
                                    stop=ko == KO - 1,
                                )
                        for half in range(2):
                            evict_half(ph[half], ot, half)
                        nc.gpsimd.dma_start(y_v[:, mt, :], ot[:])
                    else:
                        # last m-tile: run the two output halves back to back so
                        # half 0's eviction and store overlap half 1's matmuls
                        for half in range(2):
                            ps = psum.tile(
                                [P, 512], mybir.dt.float32, tag="ps", name="pf"
                            )
                            for ko in range(KO):
                                nc.tensor.matmul(
                                    ps[:],
                                    x_slice(sc, ko, mt_i),
                                    wt[ko][:, half * 512 : (half + 1) * 512],
                                    start=ko == 0,
                                    stop=ko == KO - 1,
                                )
                            evict_half(ps, ot, half)
                            nc.gpsimd.dma_start(
                                y_v[:, mt, half * 512 : (half + 1) * 512],
                                ot[:, half * 512 : (half + 1) * 512],
                            )

    nc.compile()
    return nc


def _get_nc():
    global _NC_CACHE
    if _NC_CACHE is None:
        _NC_CACHE = _build_nc()
    return _NC_CACHE


def kernel(x, W, b, A, B):
    global LAST_RESULT
    x = np.ascontiguousarray(np.asarray(x, dtype=np.float32))
    W = np.asarray(W, dtype=np.float32)
    b = np.asarray(b, dtype=np.float32)
    A = np.asarray(A, dtype=np.float32)
    B = np.asarray(B, dtype=np.float32)
    assert x.shape == (4, 4096, D) and W.shape == (O, D)
    assert b.shape == (O,) and A.shape[1] == D and B.shape[0] == O

    # Fold the LoRA update into the weight: x@W^T + s*(x@A^T)@B^T = x@(W + s*B@A)^T
    Weff = (
        W.astype(np.float64) + SCALING * (B.astype(np.float64) @ A.astype(np.float64))
    ).astype(np.float32)
    WeffT = np.ascontiguousarray(Weff.T)  # [D, O]
    bias_rep = np.ascontiguousarray(np.broadcast_to(b[None, :], (P, O)))

    xr = x.reshape(M_TOTAL, D)
    in_maps = []
    for c in range(N_CORES):
        xTc = np.ascontiguousarray(xr[c * M : (c + 1) * M].T)  # [D, M]
        in_maps.append({"xT": xTc, "wT": WeffT, "bias": bias_rep})

    nc = _get_nc()
    res = run_bass_kernel_spmd(
        nc, in_maps, core_ids=list(range(N_CORES)), trace=TRACE
    )
    LAST_RESULT = res

    out = np.concatenate([res.results[c]["y"] for c in range(N_CORES)], axis=0)
    return out.reshape(x.shape[0], x.shape[1], O)
